# revision 4
# baseline (speedup 1.0000x reference)
"""BitwiseTasNet Trainium2 kernel.

Full (unsharded) inputs in, full output out. Internally: data-parallel over
batch x time across 8 NeuronCores (4 time-shards per batch item) with halo
margins so no inter-core communication is needed. Activations and weights are
bf16 (PSUM accumulation stays fp32). The dilated depthwise conv folds its
center tap into the conv1 eviction (ev1 writes q1 = |w1c|*prelu(bn(conv1)))
so the dconv is one DVE scalar_tensor_tensor (psum = (w2/w1c)*q1R + q1C) plus
one diagonal matmul ((w0/w1c), start=False accumulate); the center-tap sign
folds into ev2's per-channel scale.
"""
import sys

sys.path.insert(0, "/opt/trn_rl_repo")

import numpy as np

import concourse.bass as bass
import concourse.mybir as mybir
import concourse.tile as tile
from concourse.bass_utils import run_bass_kernel_spmd

# Problem constants (hardcoded per contest rules).
B, T, E, D, BL, L, KT, FK, STR = 2, 64000, 256, 512, 2, 6, 3, 20, 10
EPS = 1e-5
TC = (T + 2 * FK - FK) // STR + 1  # 6403 encoder output cols
NCORES, QP = 8, 4  # 4 time-shards per batch item
NI = 1601          # interior cols per core (ceil(6403/4))
MARG = 128         # halo margin (2*63 receptive field + 2 for decoder)
NE = NI + 2 * MARG # 1857 computed cols (block 0 / encoder / decoder)
DOFF = 32          # side strip for dconv tap overhang (max dilation)
BW = 1984          # activation buffer width
XW_LEN = 19240
NL = BL * L
PCOLS_PER_LAYER = 40
NPCOL = NL * PCOLS_PER_LAYER + 8

# PSUM groups are half-width (2 banks, 4 slots) for pipeline depth. Each
# half: matmul segments (psum_off, data_off, width) and an eviction run.
# Block 1 only needs cols [63, 1794) (output valid on [126, 1731)).
HALVES0 = [
    dict(segs=[(0, 0, 512), (512, 512, 512)], ev=(0, 0, 1024)),
    dict(segs=[(0, 1024, 512), (512, 1536, 384)], ev=(0, 1024, 833)),
]
HALVES1 = [
    dict(segs=[(0, 63, 512), (512, 575, 512)], ev=(0, 63, 1024)),
    dict(segs=[(0, 1087, 512), (512, 1599, 256)], ev=(0, 1087, 707)),
]
NEW = 1920  # encoder window width (block-0 matmuls span [0, 1920))

F32 = mybir.dt.float32
F32R = mybir.dt.float32r
BF16 = mybir.dt.bfloat16
AF = mybir.ActivationFunctionType
OP = mybir.AluOpType

_built = None  # cached (module is data-independent)


def _split_multi_waits(nc, max_waits=1):
    """This walrus build accepts only one sync-wait command per instruction;
    hoist extras into standalone NoOps on the same engine just before it."""
    for fn in nc.m.functions:
        for blk in fn.blocks:
            new_insts, ctr = [], 0
            for inst in blk.instructions:
                si = inst.sync_info
                if si is not None and len(si.on_wait) > max_waits:
                    extra = si.on_wait[:-max_waits]
                    si.on_wait = si.on_wait[-max_waits:]
                    for w in extra:
                        ctr += 1
                        new_insts.append(mybir.InstNoOp(
                            name=f"{inst.name}_hw{ctr}",
                            engine=inst.engine,
                            sync_info=mybir.SyncInfo(on_wait=[w], on_update=[]),
                            bass_nofuse=True,
                        ))
                new_insts.append(inst)
            blk.instructions = new_insts


def build(loop_k=None):
    """Build the (data-independent) bass module for one core."""
    nc = bass.Bass()

    win_d = nc.dram_tensor("win", [FK, NEW], F32R, kind="ExternalInput")
    eye_d = nc.dram_tensor("eye", [128, 128], BF16, kind="ExternalInput")
    mkl_d = nc.dram_tensor("maskL", [128, 64], BF16, kind="ExternalInput")
    mkr_d = nc.dram_tensor("maskR", [128, 64], BF16, kind="ExternalInput")
    par_d = nc.dram_tensor("params", [128, NPCOL], F32, kind="ExternalInput")
    encT_d = nc.dram_tensor("encT", [FK, E], F32R, kind="ExternalInput")
    decT_d = nc.dram_tensor("decT", [128, 2, 20], BF16, kind="ExternalInput")
    w1T_d = nc.dram_tensor("w1T", [NL, 128, 2, D], BF16, kind="ExternalInput")
    w2T_d = nc.dram_tensor("w2T", [NL, 128, 4, E], BF16, kind="ExternalInput")
    y1_d = nc.dram_tensor("y1", [10, NI], F32, kind="ExternalOutput")
    y2_d = nc.dram_tensor("y2", [10, NI], F32, kind="ExternalOutput")

    with tile.TileContext(nc) as tc:
        with (
            tc.tile_pool(name="per", bufs=1) as per,
            tc.tile_pool(name="lw", bufs=3) as lw,
            tc.tile_pool(name="ps", bufs=4, space="PSUM") as psp,
        ):
            # ---- persistent tiles ----
            eye = per.tile([128, 128], BF16)
            mkl = per.tile([128, 64], BF16)
            mkr = per.tile([128, 64], BF16)
            par = per.tile([128, NPCOL], F32)
            encT = per.tile([FK, E], F32R)
            decT = per.tile([128, 2, 20], BF16)
            win = per.tile([FK, NEW], F32R)
            HI0 = per.tile([128, 2, BW], BF16)  # enc / block0 input (preserved)
            HI1 = per.tile([128, 2, BW], BF16)  # block1 input
            hP = per.tile([128, 2, BW], BF16)   # intra-block h scratch
            hF = per.tile([128, 2, BW], BF16)   # final h
            q1 = per.tile([128, 4, BW], BF16)   # |w1c|*prelu1 out (dconv input)
            v = per.tile([128, 4, BW], BF16)    # prelu2 out (conv2 input)
            warm = per.tile([128, 1], F32)

            nc.scalar.dma_start(encT[:], encT_d[:])
            nc.sync.dma_start(win[:, 0:512], win_d[:, 0:512])
            nc.sync.dma_start(win[:, 512:1024], win_d[:, 512:1024])
            nc.sync.dma_start(win[:, 1024:1536], win_d[:, 1024:1536])
            nc.sync.dma_start(win[:, 1536:NEW], win_d[:, 1536:NEW])
            nc.scalar.dma_start(par[:], par_d[:])
            nc.scalar.dma_start(eye[:], eye_d[:])
            nc.gpsimd.dma_start(mkl[:], mkl_d[:])
            nc.gpsimd.dma_start(mkr[:], mkr_d[:])
            nc.gpsimd.dma_start(decT[:], decT_d[:])

            # zero dconv overhang strips of q1 once (never re-written; the
            # computed region [DOFF, DOFF+NE) is re-filled by ev1 each layer)
            for ct in range(4):
                nc.vector.memset(q1[:, ct, 0:DOFF].bitcast(F32), 0.0)
                nc.vector.memset(q1[:, ct, DOFF + NE - 1:BW].bitcast(F32), 0.0)

            # warm the ACT table set early (parametric_relu+sigmoid+identity)
            nc.vector.memset(warm[:], 0.0)
            nc.scalar.activation(warm[:], warm[:], AF.Prelu, bias=0.0, scale=1.0, alpha=0.25)
            nc.scalar.activation(warm[:], warm[:], AF.Sigmoid, bias=0.0, scale=1.0)

            def mm_group(lhts, rhs_of, halves, evict_fn, name, mrows=128,
                         half_outer=False, start=True, pre_fn=None):
                """One output row-tile: kt-outer matmuls into per-half psum
                tiles (2 banks each), then per-half evictions. pre_fn(tile,
                hi, hv) optionally pre-loads each psum half (then start=False
                accumulates onto it)."""
                tiles = [psp.tile([128, 1024], F32, tag="ps", name=f"{name}{hi}")
                         for hi in range(len(halves))]
                if pre_fn is not None:
                    for hi, hv in enumerate(halves):
                        pre_fn(tiles[hi], hi, hv)
                nk = len(lhts)
                order = ([(hi, ki) for hi in range(len(halves)) for ki in range(nk)]
                         if half_outer else
                         [(hi, ki) for ki in range(nk) for hi in range(len(halves))])
                for hi, ki in order:
                    hv = halves[hi]
                    for (po, do, w) in hv["segs"]:
                        nc.tensor.matmul(
                            tiles[hi][0:mrows, po:po + w], lhts[ki],
                            rhs_of(ki, do, w),
                            start=(ki == 0 and start), stop=(ki == nk - 1),
                            skip_group_check=True,
                        )
                    if ki == nk - 1 and half_outer:
                        (po, do, w) = hv["ev"]
                        evict_fn(tiles[hi], po, do, w)
                if not half_outer:
                    for hi, hv in enumerate(halves):
                        (po, do, w) = hv["ev"]
                        evict_fn(tiles[hi], po, do, w)

            def emit_body():
                # ---- encoder: enc = encT.T @ win (K=20), evict with +enc_b ----
                for mt in range(2):
                    ebias = par[:, NL * PCOLS_PER_LAYER + mt: NL * PCOLS_PER_LAYER + mt + 1]
                    def enc_evict(ps, po, do, w, mt=mt, ebias=ebias):
                        nc.scalar.activation(
                            HI0[:, mt, DOFF + do:DOFF + do + w], ps[:, po:po + w],
                            AF.Identity, bias=ebias, scale=1.0)
                    mm_group(
                        [encT[:, mt * 128:(mt + 1) * 128]],
                        lambda ki, do, w: win[:, do:do + w],
                        HALVES0, enc_evict, f"enc{mt}")

                # ---- TCN ----
                hcur = HI0
                for b in range(BL):
                    halves = HALVES0 if b == 0 else HALVES1
                    resid = hcur
                    for l in range(L):
                        li = b * L + l
                        base = li * PCOLS_PER_LAYER
                        dil = 1 << l

                        w1t = lw.tile([128, 2, D], BF16, tag="w1t")
                        w2t = lw.tile([128, 4, E], BF16, tag="w2t")
                        dg = lw.tile([128, 4, 128], BF16, tag="dg")
                        nc.sync.dma_start(w1t[:], w1T_d[li])
                        nc.gpsimd.dma_start(w2t[:], w2T_d[li])
                        # diag matrices for the left tap: dg[:, ct, :] = eye*d0
                        for ct in range(4):
                            nc.vector.tensor_scalar_mul(
                                dg[:, ct, :], eye[:],
                                par[:, base + 8 + ct: base + 9 + ct],
                            )

                        # conv1 (E->D) + Prelu/BN eviction into q1 = |w1c|*p
                        for ct in range(4):
                            def ev1(ps, po, do, w, ct=ct):
                                nc.scalar.activation(
                                    q1[:, ct, DOFF + do:DOFF + do + w], ps[:, po:po + w],
                                    AF.Prelu,
                                    bias=par[:, base + 4 + ct: base + 5 + ct],
                                    scale=par[:, base + ct: base + 1 + ct],
                                    alpha=par[:, base + 38: base + 39],
                                )
                            mm_group(
                                [w1t[:, kt, ct * 128:(ct + 1) * 128] for kt in range(2)],
                                lambda ki, do, w: hcur[:, ki, DOFF + do:DOFF + do + w],
                                halves, ev1, f"c1_{ct}_")
                            # zero-pad masks on the dconv input (per-core data),
                            # then fill tap-reachable pad cols with -C1*|w1c| so
                            # the folded dconv bias is exact at true tensor edges
                            nc.vector.tensor_mul(
                                q1[:, ct, 96:160], q1[:, ct, 96:160], mkl[:])
                            nc.vector.tensor_scalar_add(
                                q1[:, ct, 128:160], q1[:, ct, 128:160],
                                par[:, base + 28 + ct: base + 29 + ct])
                            nc.vector.tensor_mul(
                                q1[:, ct, 1760:1824], q1[:, ct, 1760:1824], mkr[:])
                            nc.vector.tensor_scalar_add(
                                q1[:, ct, 1760:1792], q1[:, ct, 1760:1792],
                                par[:, base + 32 + ct: base + 33 + ct])

                        # depthwise dilated conv: DVE stt pre-loads each psum
                        # half with d2*q1R + q1C, then one diagonal matmul
                        # accumulates d0*q1L (start=False); Act evicts with
                        # Prelu/BN (sign of w1c folded into the scale) into v
                        for ct in range(4):
                            def pre2(pt, hi, hv, ct=ct):
                                # cover the full matmul span so start=False
                                # accumulates onto stt-written psum only
                                po = hv["segs"][0][0]
                                do = hv["segs"][0][1]
                                w = hv["segs"][-1][0] + hv["segs"][-1][2] - po
                                nc.vector.scalar_tensor_tensor(
                                    pt[:, po:po + w],
                                    q1[:, ct, DOFF + dil + do:DOFF + dil + do + w],
                                    par[:, base + 12 + ct: base + 13 + ct],
                                    q1[:, ct, DOFF + do:DOFF + do + w],
                                    op0=OP.mult, op1=OP.add,
                                )
                            def ev2(ps, po, do, w, ct=ct):
                                nc.scalar.activation(
                                    v[:, ct, DOFF + do:DOFF + do + w], ps[:, po:po + w],
                                    AF.Prelu,
                                    bias=par[:, base + 24 + ct: base + 25 + ct],
                                    scale=par[:, base + 20 + ct: base + 21 + ct],
                                    alpha=par[:, base + 39: base + 40],
                                )
                            mm_group(
                                [dg[:, ct, :]],
                                lambda ki, do, w, ct=ct: q1[:, ct, DOFF - dil + do:
                                                            DOFF - dil + do + w],
                                halves, ev2, f"dc_{ct}_", start=False, pre_fn=pre2)

                        # conv2 (D->E) + h update
                        last = (l == L - 1)
                        hnext = (HI1 if b == 0 else hF) if last else hP
                        for ct2 in range(2):
                            eb = par[:, base + 36 + ct2: base + 37 + ct2]
                            def ev3(ps, po, do, w, ct2=ct2, eb=eb, last=last):
                                if last:
                                    nc.vector.scalar_tensor_tensor(
                                        hnext[:, ct2, DOFF + do:DOFF + do + w],
                                        ps[:, po:po + w], eb,
                                        resid[:, ct2, DOFF + do:DOFF + do + w],
                                        op0=OP.add, op1=OP.add,
                                    )
                                else:
                                    nc.vector.tensor_scalar_add(
                                        hnext[:, ct2, DOFF + do:DOFF + do + w],
                                        ps[:, po:po + w], eb)
                            mm_group(
                                [w2t[:, kt, ct2 * 128:(ct2 + 1) * 128] for kt in range(4)],
                                lambda ki, do, w: v[:, ki, DOFF + do:DOFF + do + w],
                                halves, ev3, f"c2_{ct2}_")
                        hcur = hnext

                # ---- mask + decoder (full range, HALVES0 layout) ----
                sig = q1  # reuse
                mk = v
                for ct2 in range(2):
                    for (c0, c1) in ((0, 512), (512, 1024), (1024, 1536), (1536, NE)):
                        nc.scalar.activation(
                            sig[:, ct2, DOFF + c0:DOFF + c1], hF[:, ct2, DOFF + c0:DOFF + c1],
                            AF.Sigmoid, bias=0.0, scale=1.0)
                        nc.vector.tensor_mul(
                            mk[:, ct2, DOFF + c0:DOFF + c1],
                            HI0[:, ct2, DOFF + c0:DOFF + c1],
                            sig[:, ct2, DOFF + c0:DOFF + c1])
                dsb = per.tile([20, NE], F32)
                def evd(ps, po, do, w):
                    nc.scalar.activation(
                        dsb[:, do:do + w], ps[0:20, po:po + w], AF.Copy)
                mm_group(
                    [decT[:, kt, :] for kt in range(2)],
                    lambda ki, do, w: mk[:, ki, DOFF + do:DOFF + do + w],
                    HALVES0, evd, "dec_", mrows=20, half_outer=True)
                # out[10m+r] = P1[r, m+MARG+2] + P2[r, m+MARG+1]  (host adds them)
                ys = 1024 - MARG - 2  # first-piece width aligned to dsb piece A
                nc.sync.dma_start(y1_d[:, 0:ys], dsb[0:10, MARG + 2:1024])
                nc.sync.dma_start(y2_d[:, 0:ys + 1], dsb[10:20, MARG + 1:1024])
                nc.sync.dma_start(y1_d[:, ys:NI], dsb[0:10, 1024:MARG + 2 + NI])
                nc.sync.dma_start(y2_d[:, ys + 1:NI], dsb[10:20, 1024:MARG + 1 + NI])


            if loop_k is None:
                emit_body()
            else:
                with tc.For_i(0, loop_k):
                    emit_body()

    _split_multi_waits(nc)
    return nc


def _host_prep(inputs):
    """Per-core in_maps + assembly metadata from full inputs."""
    import ml_dtypes
    f32 = np.float32
    bf16 = ml_dtypes.bfloat16
    x = np.asarray(inputs["x"], f32)
    enc_w = np.asarray(inputs["enc_w"], f32)
    enc_b = np.asarray(inputs["enc_b"], f32)
    w1 = np.asarray(inputs["w1"], f32)
    b1 = np.asarray(inputs["b1"], f32)
    a1 = np.asarray(inputs["a1"], f32)
    g1 = np.asarray(inputs["g1"], f32)
    be1 = np.asarray(inputs["be1"], f32)
    m1 = np.asarray(inputs["m1"], f32)
    v1 = np.asarray(inputs["v1"], f32)
    wd = np.asarray(inputs["wd"], f32)
    bd = np.asarray(inputs["bd"], f32)
    a2 = np.asarray(inputs["a2"], f32)
    g2 = np.asarray(inputs["g2"], f32)
    be2 = np.asarray(inputs["be2"], f32)
    m2 = np.asarray(inputs["m2"], f32)
    v2 = np.asarray(inputs["v2"], f32)
    w2 = np.asarray(inputs["w2"], f32)
    b2 = np.asarray(inputs["b2"], f32)
    dec_w = np.asarray(inputs["dec_w"], f32)
    dec_b = np.asarray(inputs["dec_b"], f32)

    eye = np.eye(128, dtype=f32).astype(bf16)
    encT = np.ascontiguousarray(enc_w[:, 0, :].T)  # [FK, E]
    decT = np.zeros((128, 2, 20), f32)
    for kt in range(2):
        decT[:, kt, :] = dec_w[kt * 128:(kt + 1) * 128, 0, :]
    decT = decT.astype(bf16)

    w1T = np.zeros((NL, 128, 2, D), f32)
    w2T = np.zeros((NL, 128, 4, E), f32)
    C1s = np.zeros((NL, D), np.float64)  # (-c1)*|w1c| edge-fix values
    par_shared = np.zeros((128, NPCOL), f32)
    for b in range(BL):
        for l in range(L):
            li = b * L + l
            base = li * PCOLS_PER_LAYER
            w1bl = w1[b, l, :, :, 0].astype(np.float64)  # [D, E]
            w2bl = w2[b, l, :, :, 0].astype(np.float64)  # [E, D]
            for kt in range(2):
                w1T[li, :, kt, :] = w1bl.T[kt * 128:(kt + 1) * 128, :]
            for kt in range(4):
                w2T[li, :, kt, :] = w2bl.T[kt * 128:(kt + 1) * 128, :]
            s1 = g1[b, l] / np.sqrt(v1[b, l].astype(np.float64) + EPS)
            c1 = be1[b, l] - m1[b, l] * s1
            s2 = g2[b, l] / np.sqrt(v2[b, l].astype(np.float64) + EPS)
            c2 = be2[b, l] - m2[b, l] * s2
            w0, w1c, w2c = (wd[b, l, :, 0, k].astype(np.float64) for k in range(3))
            aw = np.abs(w1c)
            sg = np.sign(w1c)
            C1s[li] = (-c1) * aw
            bias2p = bd[b, l] + c1 * (w0 + w1c + w2c)
            b2pp = b2[b, l] + w2bl @ c2

            def col(idx, vals512):
                par_shared[:, base + idx:base + idx + 4] = np.asarray(
                    vals512, f32).reshape(4, 128).T
            col(0, s1 * aw)
            col(4, s1 * b1[b, l] * aw)
            col(8, w0 / w1c)   # d0 (left-tap diag values)
            col(12, w2c / w1c) # d2 (right-tap stt scale)
            col(20, s2 * sg)
            col(24, s2 * bias2p)
            # 28..35: fixL/fixR are per-core (filled later)
            par_shared[:, base + 36:base + 38] = np.asarray(
                b2pp, f32).reshape(2, 128).T
            par_shared[:, base + 38] = a1[b, l]
            par_shared[:, base + 39] = a2[b, l]
    par_shared[:, NL * PCOLS_PER_LAYER:NL * PCOLS_PER_LAYER + 2] = \
        enc_b.reshape(2, 128).T

    w1T = w1T.astype(bf16)
    w2T = w2T.astype(bf16)

    in_maps = []
    ones64 = np.ones((128, 64), f32).astype(bf16)
    zeros64 = np.zeros((128, 64), f32).astype(bf16)
    for core in range(NCORES):
        bb, q = divmod(core, QP)
        xbase = 16010 * q - 1300
        xw = np.zeros(XW_LEN, f32)
        lo, hi = max(0, xbase), min(T, xbase + XW_LEN)
        if hi > lo:
            xw[lo - xbase:hi - xbase] = x[bb, 0, lo:hi]
        # im2col on host: win[k, j] = xw[10j + k]
        winm = np.lib.stride_tricks.as_strided(
            xw, shape=(1920, FK), strides=(40, 4)).T.copy()
        par = par_shared.copy()
        left, right = (q == 0), (q == QP - 1)
        for li in range(NL):
            base = li * PCOLS_PER_LAYER
            fixL = C1s[li] if left else np.zeros(D)
            fixR = C1s[li] if right else np.zeros(D)
            par[:, base + 28:base + 32] = np.asarray(fixL, f32).reshape(4, 128).T
            par[:, base + 32:base + 36] = np.asarray(fixR, f32).reshape(4, 128).T
        in_maps.append(dict(
            win=winm, eye=eye,
            maskL=(zeros64 if left else ones64),
            maskR=(zeros64 if right else ones64),
            params=par, encT=encT, decT=decT, w1T=w1T, w2T=w2T,
        ))
    return in_maps, float(dec_b[0])


def kernel(**inputs):
    global _built
    if _built is None:
        _built = build()
    nc = _built
    in_maps, decb = _host_prep(inputs)
    res = run_bass_kernel_spmd(nc, in_maps, core_ids=list(range(NCORES)))
    out = np.zeros((B, 1, T), np.float32)
    for core in range(NCORES):
        bb, q = divmod(core, QP)
        seg = (res.results[core]["y1"] + res.results[core]["y2"]).T.reshape(-1)
        t0 = q * NI * STR
        n = min(T - t0, NI * STR)
        out[bb, 0, t0:t0 + n] = seg[:n] + decb
    return out


# revision 11
# speedup vs baseline: 1.0305x; 1.0305x over previous
"""BitwiseTasNet Trainium2 kernel.

Full (unsharded) inputs in, full output out. Internally: data-parallel over
batch x time across 8 NeuronCores (4 time-shards per batch item) with halo
margins so no inter-core communication is needed. Activations and weights are
bf16 (PSUM accumulation stays fp32). The dilated depthwise conv folds its
center tap into the conv1 eviction (ev1 writes q1 = |w1c|*prelu(bn(conv1)))
so the dconv is one DVE scalar_tensor_tensor (psum = (w2/w1c)*q1R + q1C) plus
one diagonal matmul ((w0/w1c), start=False accumulate); the center-tap sign
folds into ev2's per-channel scale.
"""
import sys

sys.path.insert(0, "/opt/trn_rl_repo")

import numpy as np

import concourse.bass as bass
import concourse.mybir as mybir
import concourse.tile as tile
from concourse.bass_utils import run_bass_kernel_spmd

# Problem constants (hardcoded per contest rules).
B, T, E, D, BL, L, KT, FK, STR = 2, 64000, 256, 512, 2, 6, 3, 20, 10
EPS = 1e-5
TC = (T + 2 * FK - FK) // STR + 1  # 6403 encoder output cols
NCORES, QP = 8, 4  # 4 time-shards per batch item
NI = 1601          # interior cols per core (ceil(6403/4))
MARG = 128         # halo margin (2*63 receptive field + 2 for decoder)
NE = NI + 2 * MARG # 1857 computed cols (block 0 / encoder / decoder)
DOFF = 32          # side strip for dconv tap overhang (max dilation)
BW = 1984          # activation buffer width
XW_LEN = 19240
NL = BL * L
PCOLS_PER_LAYER = 40
NPCOL = NL * PCOLS_PER_LAYER + 8

# PSUM groups are half-width (2 banks, 4 slots) for pipeline depth. Each
# half: matmul segments (psum_off, data_off, width) and an eviction run.
NEW = 1920  # encoder window width (block-0 matmuls span [0, 1920))


def make_halves(a, b):
    """Split the frame interval [a, b) into two psum halves of <=1024 cols,
    each tiled by <=512-col matmul segments at bank-aligned psum offsets."""
    w = b - a
    halves = []
    for off in range(0, w, 1024):
        hw = min(1024, w - off)
        segs = []
        for so in range(0, hw, 512):
            segs.append((so, a + off + so, min(512, hw - so)))
        halves.append(dict(segs=segs, ev=(0, a + off, hw)))
    return halves


def layer_intervals():
    """Per (block, layer): conv1-out interval and dconv/conv2-out interval
    in the NE frame, shrinking with the remaining receptive field."""
    res = {}
    cur = (129, 1731)  # decoder/mask cols
    for bI in reversed(range(BL)):
        for l in reversed(range(L)):
            dil = 1 << l
            out = cur
            c1 = (out[0] - dil, out[1] + dil)
            res[(bI, l)] = (c1, out)
            cur = c1
    return res


INTERVALS = layer_intervals()
# encoder runs in f32r, which requires 128-multiple segment widths at
# bank-aligned psum offsets; evictions still cover exactly [0, NE)
HALVES0 = [
    dict(segs=[(0, 0, 512), (512, 512, 512)], ev=(0, 0, 1024)),
    dict(segs=[(0, 1024, 512), (512, 1536, 384)], ev=(0, 1024, 833)),
]

F32 = mybir.dt.float32
F32R = mybir.dt.float32r
BF16 = mybir.dt.bfloat16
AF = mybir.ActivationFunctionType
OP = mybir.AluOpType

_built = None  # cached (module is data-independent)


def _split_multi_waits(nc, max_waits=1):
    """This walrus build accepts only one sync-wait command per instruction;
    hoist extras into standalone NoOps on the same engine just before it."""
    for fn in nc.m.functions:
        for blk in fn.blocks:
            new_insts, ctr = [], 0
            for inst in blk.instructions:
                si = inst.sync_info
                if si is not None and len(si.on_wait) > max_waits:
                    extra = si.on_wait[:-max_waits]
                    si.on_wait = si.on_wait[-max_waits:]
                    for w in extra:
                        ctr += 1
                        new_insts.append(mybir.InstNoOp(
                            name=f"{inst.name}_hw{ctr}",
                            engine=inst.engine,
                            sync_info=mybir.SyncInfo(on_wait=[w], on_update=[]),
                            bass_nofuse=True,
                        ))
                new_insts.append(inst)
            blk.instructions = new_insts


def build(loop_k=None):
    """Build the (data-independent) bass module for one core."""
    nc = bass.Bass()

    win_d = nc.dram_tensor("win", [FK, NEW], F32R, kind="ExternalInput")
    eye_d = nc.dram_tensor("eye", [128, 128], BF16, kind="ExternalInput")
    mkl_d = nc.dram_tensor("maskL", [128, 64], BF16, kind="ExternalInput")
    mkr_d = nc.dram_tensor("maskR", [128, 64], BF16, kind="ExternalInput")
    par_d = nc.dram_tensor("params", [128, NPCOL], F32, kind="ExternalInput")
    encT_d = nc.dram_tensor("encT", [FK, E], F32R, kind="ExternalInput")
    decT_d = nc.dram_tensor("decT", [128, 2, 20], BF16, kind="ExternalInput")
    w1T_d = nc.dram_tensor("w1T", [NL, 128, 2, D], BF16, kind="ExternalInput")
    w2T_d = nc.dram_tensor("w2T", [NL, 128, 4, E], BF16, kind="ExternalInput")
    y1_d = nc.dram_tensor("y1", [10, NI], F32, kind="ExternalOutput")
    y2_d = nc.dram_tensor("y2", [10, NI], F32, kind="ExternalOutput")

    with tile.TileContext(nc) as tc:
        with (
            tc.tile_pool(name="per", bufs=1) as per,
            tc.tile_pool(name="lw", bufs=3) as lw,
            tc.tile_pool(name="ps", bufs=4, space="PSUM") as psp,
        ):
            # ---- persistent tiles ----
            eye = per.tile([128, 128], BF16)
            mkl = per.tile([128, 64], BF16)
            mkr = per.tile([128, 64], BF16)
            par = per.tile([128, NPCOL], F32)
            encT = per.tile([FK, E], F32R)
            decT = per.tile([128, 2, 20], BF16)
            win = per.tile([FK, NEW], F32R)
            HI0 = per.tile([128, 2, BW], BF16)  # enc / block0 input (preserved)
            HI1 = per.tile([128, 2, BW], BF16)  # block1 input
            hP = per.tile([128, 2, BW], BF16)   # intra-block h scratch
            hF = per.tile([128, 2, BW], BF16)   # final h
            q1 = per.tile([128, 4, BW], BF16)   # |w1c|*prelu1 out (dconv input)
            v = per.tile([128, 4, BW], BF16)    # prelu2 out (conv2 input)
            warm = per.tile([128, 1], F32)

            nc.scalar.dma_start(encT[:], encT_d[:])
            nc.sync.dma_start(win[:, 0:512], win_d[:, 0:512])
            nc.sync.dma_start(win[:, 512:1024], win_d[:, 512:1024])
            nc.sync.dma_start(win[:, 1024:1536], win_d[:, 1024:1536])
            nc.sync.dma_start(win[:, 1536:NEW], win_d[:, 1536:NEW])
            nc.scalar.dma_start(par[:], par_d[:])
            nc.scalar.dma_start(eye[:], eye_d[:])
            nc.gpsimd.dma_start(mkl[:], mkl_d[:])
            nc.gpsimd.dma_start(mkr[:], mkr_d[:])
            nc.gpsimd.dma_start(decT[:], decT_d[:])

            # zero dconv overhang strips of q1 once (never re-written; the
            # computed region [DOFF, DOFF+NE) is re-filled by ev1 each layer)
            for ct in range(4):
                nc.vector.memset(q1[:, ct, 0:DOFF].bitcast(F32), 0.0)
                nc.vector.memset(q1[:, ct, DOFF + NE - 1:BW].bitcast(F32), 0.0)

            # warm the ACT table set early (parametric_relu+sigmoid+identity)
            nc.vector.memset(warm[:], 0.0)
            nc.scalar.activation(warm[:], warm[:], AF.Prelu, bias=0.0, scale=1.0, alpha=0.25)
            nc.scalar.activation(warm[:], warm[:], AF.Sigmoid, bias=0.0, scale=1.0)

            def mm_group(lhts, rhs_of, halves, evict_fn, name, mrows=128,
                         half_outer=False, start=True, pre_fn=None):
                """One output row-tile: kt-outer matmuls into per-half psum
                tiles (2 banks each), then per-half evictions. pre_fn(tile,
                hi, hv) optionally pre-loads each psum half (then start=False
                accumulates onto it)."""
                tiles = [psp.tile([128, 1024], F32, tag="ps", name=f"{name}{hi}")
                         for hi in range(len(halves))]
                if pre_fn is not None:
                    for hi, hv in enumerate(halves):
                        pre_fn(tiles[hi], hi, hv)
                nk = len(lhts)
                order = ([(hi, ki) for hi in range(len(halves)) for ki in range(nk)]
                         if half_outer else
                         [(hi, ki) for ki in range(nk) for hi in range(len(halves))])
                for hi, ki in order:
                    hv = halves[hi]
                    for (po, do, w) in hv["segs"]:
                        nc.tensor.matmul(
                            tiles[hi][0:mrows, po:po + w], lhts[ki],
                            rhs_of(ki, do, w),
                            start=(ki == 0 and start), stop=(ki == nk - 1),
                            skip_group_check=True,
                        )
                    if ki == nk - 1 and half_outer:
                        (po, do, w) = hv["ev"]
                        evict_fn(tiles[hi], po, do, w)
                if not half_outer:
                    for hi, hv in enumerate(halves):
                        (po, do, w) = hv["ev"]
                        evict_fn(tiles[hi], po, do, w)

            def emit_body():
                # ---- encoder: enc = encT.T @ win (K=20), evict on DVE ----
                for mt in range(2):
                    ebias = par[:, NL * PCOLS_PER_LAYER + mt: NL * PCOLS_PER_LAYER + mt + 1]
                    def enc_evict(ps, po, do, w, mt=mt, ebias=ebias):
                        nc.vector.tensor_scalar_add(
                            HI0[:, mt, DOFF + do:DOFF + do + w], ps[:, po:po + w],
                            ebias)
                    mm_group(
                        [encT[:, mt * 128:(mt + 1) * 128]],
                        lambda ki, do, w: win[:, do:do + w],
                        HALVES0, enc_evict, f"enc{mt}")

                # ---- TCN ----
                hcur = HI0
                for b in range(BL):
                    resid = hcur
                    for l in range(L):
                        li = b * L + l
                        base = li * PCOLS_PER_LAYER
                        dil = 1 << l
                        (c1a, c1b), (oa, ob) = INTERVALS[(b, l)]
                        hv1 = make_halves(c1a, c1b)
                        hv2 = make_halves(oa, ob)

                        w1t = lw.tile([128, 2, D], BF16, tag="w1t")
                        w2t = lw.tile([128, 4, E], BF16, tag="w2t")
                        dg = lw.tile([128, 4, 128], BF16, tag="dg")
                        nc.sync.dma_start(w1t[:], w1T_d[li])
                        nc.gpsimd.dma_start(w2t[:], w2T_d[li])
                        # diag matrices for the left tap: dg[:, ct, :] = eye*d0
                        for ct in range(4):
                            nc.vector.tensor_scalar_mul(
                                dg[:, ct, :], eye[:],
                                par[:, base + 8 + ct: base + 9 + ct],
                            )

                        # conv1 (E->D) + Prelu/BN eviction into q1 = |w1c|*p
                        for ct in range(4):
                            def ev1(ps, po, do, w, ct=ct):
                                nc.scalar.activation(
                                    q1[:, ct, DOFF + do:DOFF + do + w], ps[:, po:po + w],
                                    AF.Prelu,
                                    bias=par[:, base + 4 + ct: base + 5 + ct],
                                    scale=par[:, base + ct: base + 1 + ct],
                                    alpha=par[:, base + 38: base + 39],
                                )
                            mm_group(
                                [w1t[:, kt, ct * 128:(ct + 1) * 128] for kt in range(2)],
                                lambda ki, do, w: hcur[:, ki, DOFF + do:DOFF + do + w],
                                hv1, ev1, f"c1_{ct}_")
                            # zero-pad masks on the dconv input (per-core data),
                            # then fill tap-reachable pad cols with -C1*|w1c| so
                            # the folded dconv bias is exact at true tensor edges
                            nc.vector.tensor_mul(
                                q1[:, ct, 96:160], q1[:, ct, 96:160], mkl[:])
                            nc.vector.tensor_scalar_add(
                                q1[:, ct, 128:160], q1[:, ct, 128:160],
                                par[:, base + 28 + ct: base + 29 + ct])
                            nc.vector.tensor_mul(
                                q1[:, ct, 1760:1824], q1[:, ct, 1760:1824], mkr[:])
                            nc.vector.tensor_scalar_add(
                                q1[:, ct, 1760:1792], q1[:, ct, 1760:1792],
                                par[:, base + 32 + ct: base + 33 + ct])

                        # depthwise dilated conv: DVE stt pre-loads each psum
                        # half with d2*q1R + q1C, then one diagonal matmul
                        # accumulates d0*q1L (start=False); Act evicts with
                        # Prelu/BN (sign of w1c folded into the scale) into v
                        for ct in range(4):
                            def pre2(pt, hi, hv, ct=ct):
                                (po, do, w) = hv["ev"]
                                nc.vector.scalar_tensor_tensor(
                                    pt[:, po:po + w],
                                    q1[:, ct, DOFF + dil + do:DOFF + dil + do + w],
                                    par[:, base + 12 + ct: base + 13 + ct],
                                    q1[:, ct, DOFF + do:DOFF + do + w],
                                    op0=OP.mult, op1=OP.add,
                                )
                            def ev2(ps, po, do, w, ct=ct):
                                nc.scalar.activation(
                                    v[:, ct, DOFF + do:DOFF + do + w], ps[:, po:po + w],
                                    AF.Prelu,
                                    bias=par[:, base + 24 + ct: base + 25 + ct],
                                    scale=par[:, base + 20 + ct: base + 21 + ct],
                                    alpha=par[:, base + 39: base + 40],
                                )
                            mm_group(
                                [dg[:, ct, :]],
                                lambda ki, do, w, ct=ct: q1[:, ct, DOFF - dil + do:
                                                            DOFF - dil + do + w],
                                hv2, ev2, f"dc_{ct}_", start=False, pre_fn=pre2)

                        # conv2 (D->E) + h update
                        last = (l == L - 1)
                        hnext = (HI1 if b == 0 else hF) if last else hP
                        for ct2 in range(2):
                            eb = par[:, base + 36 + ct2: base + 37 + ct2]
                            def ev3(ps, po, do, w, ct2=ct2, eb=eb, last=last):
                                if last:
                                    nc.vector.scalar_tensor_tensor(
                                        hnext[:, ct2, DOFF + do:DOFF + do + w],
                                        ps[:, po:po + w], eb,
                                        resid[:, ct2, DOFF + do:DOFF + do + w],
                                        op0=OP.add, op1=OP.add,
                                    )
                                else:
                                    nc.vector.tensor_scalar_add(
                                        hnext[:, ct2, DOFF + do:DOFF + do + w],
                                        ps[:, po:po + w], eb)
                            mm_group(
                                [w2t[:, kt, ct2 * 128:(ct2 + 1) * 128] for kt in range(4)],
                                lambda ki, do, w: v[:, ki, DOFF + do:DOFF + do + w],
                                hv2, ev3, f"c2_{ct2}_")
                        hcur = hnext

                # ---- mask + decoder (cols [128, 1731) only) ----
                sig = q1  # reuse
                mk = v
                for ct2 in range(2):
                    for (c0, c1) in ((129, 529), (529, 929), (929, 1331), (1331, 1731)):
                        nc.scalar.activation(
                            sig[:, ct2, DOFF + c0:DOFF + c1], hF[:, ct2, DOFF + c0:DOFF + c1],
                            AF.Sigmoid, bias=0.0, scale=1.0)
                        nc.vector.tensor_mul(
                            mk[:, ct2, DOFF + c0:DOFF + c1],
                            HI0[:, ct2, DOFF + c0:DOFF + c1],
                            sig[:, ct2, DOFF + c0:DOFF + c1])
                dsb = per.tile([20, NE], F32)
                def evd(ps, po, do, w):
                    nc.scalar.activation(
                        dsb[:, do:do + w], ps[0:20, po:po + w], AF.Copy)
                mm_group(
                    [decT[:, kt, :] for kt in range(2)],
                    lambda ki, do, w: mk[:, ki, DOFF + do:DOFF + do + w],
                    make_halves(129, 1731), evd, "dec_", mrows=20, half_outer=True)
                # out[10m+r] = P1[r, m+MARG+2] + P2[r, m+MARG+1]  (host adds them)
                PB = 129 + 1024  # dsb piece-A boundary
                ys = PB - MARG - 2  # first-piece width aligned to dsb piece A
                nc.sync.dma_start(y1_d[:, 0:ys], dsb[0:10, MARG + 2:PB])
                nc.sync.dma_start(y2_d[:, 0:ys + 1], dsb[10:20, MARG + 1:PB])
                nc.sync.dma_start(y1_d[:, ys:NI], dsb[0:10, PB:MARG + 2 + NI])
                nc.sync.dma_start(y2_d[:, ys + 1:NI], dsb[10:20, PB:MARG + 1 + NI])


            if loop_k is None:
                emit_body()
            else:
                with tc.For_i(0, loop_k):
                    emit_body()

    _split_multi_waits(nc)
    return nc


def _host_prep(inputs):
    """Per-core in_maps + assembly metadata from full inputs."""
    import ml_dtypes
    f32 = np.float32
    bf16 = ml_dtypes.bfloat16
    x = np.asarray(inputs["x"], f32)
    enc_w = np.asarray(inputs["enc_w"], f32)
    enc_b = np.asarray(inputs["enc_b"], f32)
    w1 = np.asarray(inputs["w1"], f32)
    b1 = np.asarray(inputs["b1"], f32)
    a1 = np.asarray(inputs["a1"], f32)
    g1 = np.asarray(inputs["g1"], f32)
    be1 = np.asarray(inputs["be1"], f32)
    m1 = np.asarray(inputs["m1"], f32)
    v1 = np.asarray(inputs["v1"], f32)
    wd = np.asarray(inputs["wd"], f32)
    bd = np.asarray(inputs["bd"], f32)
    a2 = np.asarray(inputs["a2"], f32)
    g2 = np.asarray(inputs["g2"], f32)
    be2 = np.asarray(inputs["be2"], f32)
    m2 = np.asarray(inputs["m2"], f32)
    v2 = np.asarray(inputs["v2"], f32)
    w2 = np.asarray(inputs["w2"], f32)
    b2 = np.asarray(inputs["b2"], f32)
    dec_w = np.asarray(inputs["dec_w"], f32)
    dec_b = np.asarray(inputs["dec_b"], f32)

    eye = np.eye(128, dtype=f32).astype(bf16)
    encT = np.ascontiguousarray(enc_w[:, 0, :].T)  # [FK, E]
    decT = np.zeros((128, 2, 20), f32)
    for kt in range(2):
        decT[:, kt, :] = dec_w[kt * 128:(kt + 1) * 128, 0, :]
    decT = decT.astype(bf16)

    w1T = np.zeros((NL, 128, 2, D), f32)
    w2T = np.zeros((NL, 128, 4, E), f32)
    C1s = np.zeros((NL, D), np.float64)  # (-c1)*|w1c| edge-fix values
    par_shared = np.zeros((128, NPCOL), f32)
    for b in range(BL):
        for l in range(L):
            li = b * L + l
            base = li * PCOLS_PER_LAYER
            w1bl = w1[b, l, :, :, 0].astype(np.float64)  # [D, E]
            w2bl = w2[b, l, :, :, 0].astype(np.float64)  # [E, D]
            for kt in range(2):
                w1T[li, :, kt, :] = w1bl.T[kt * 128:(kt + 1) * 128, :]
            for kt in range(4):
                w2T[li, :, kt, :] = w2bl.T[kt * 128:(kt + 1) * 128, :]
            s1 = g1[b, l] / np.sqrt(v1[b, l].astype(np.float64) + EPS)
            c1 = be1[b, l] - m1[b, l] * s1
            s2 = g2[b, l] / np.sqrt(v2[b, l].astype(np.float64) + EPS)
            c2 = be2[b, l] - m2[b, l] * s2
            w0, w1c, w2c = (wd[b, l, :, 0, k].astype(np.float64) for k in range(3))
            aw = np.abs(w1c)
            sg = np.sign(w1c)
            C1s[li] = (-c1) * aw
            bias2p = bd[b, l] + c1 * (w0 + w1c + w2c)
            b2pp = b2[b, l] + w2bl @ c2

            def col(idx, vals512):
                par_shared[:, base + idx:base + idx + 4] = np.asarray(
                    vals512, f32).reshape(4, 128).T
            col(0, s1 * aw)
            col(4, s1 * b1[b, l] * aw)
            col(8, w0 / w1c)   # d0 (left-tap diag values)
            col(12, w2c / w1c) # d2 (right-tap stt scale)
            col(20, s2 * sg)
            col(24, s2 * bias2p)
            # 28..35: fixL/fixR are per-core (filled later)
            par_shared[:, base + 36:base + 38] = np.asarray(
                b2pp, f32).reshape(2, 128).T
            par_shared[:, base + 38] = a1[b, l]
            par_shared[:, base + 39] = a2[b, l]
    par_shared[:, NL * PCOLS_PER_LAYER:NL * PCOLS_PER_LAYER + 2] = \
        enc_b.reshape(2, 128).T

    w1T = w1T.astype(bf16)
    w2T = w2T.astype(bf16)

    in_maps = []
    ones64 = np.ones((128, 64), f32).astype(bf16)
    zeros64 = np.zeros((128, 64), f32).astype(bf16)
    for core in range(NCORES):
        bb, q = divmod(core, QP)
        xbase = 16010 * q - 1300
        xw = np.zeros(XW_LEN, f32)
        lo, hi = max(0, xbase), min(T, xbase + XW_LEN)
        if hi > lo:
            xw[lo - xbase:hi - xbase] = x[bb, 0, lo:hi]
        # im2col on host: win[k, j] = xw[10j + k]
        winm = np.lib.stride_tricks.as_strided(
            xw, shape=(1920, FK), strides=(40, 4)).T.copy()
        par = par_shared.copy()
        left, right = (q == 0), (q == QP - 1)
        for li in range(NL):
            base = li * PCOLS_PER_LAYER
            fixL = C1s[li] if left else np.zeros(D)
            fixR = C1s[li] if right else np.zeros(D)
            par[:, base + 28:base + 32] = np.asarray(fixL, f32).reshape(4, 128).T
            par[:, base + 32:base + 36] = np.asarray(fixR, f32).reshape(4, 128).T
        in_maps.append(dict(
            win=winm, eye=eye,
            maskL=(zeros64 if left else ones64),
            maskR=(zeros64 if right else ones64),
            params=par, encT=encT, decT=decT, w1T=w1T, w2T=w2T,
        ))
    return in_maps, float(dec_b[0])


def kernel(**inputs):
    global _built
    if _built is None:
        _built = build()
    nc = _built
    in_maps, decb = _host_prep(inputs)
    res = run_bass_kernel_spmd(nc, in_maps, core_ids=list(range(NCORES)))
    out = np.zeros((B, 1, T), np.float32)
    for core in range(NCORES):
        bb, q = divmod(core, QP)
        seg = (res.results[core]["y1"] + res.results[core]["y2"]).T.reshape(-1)
        t0 = q * NI * STR
        n = min(T - t0, NI * STR)
        out[bb, 0, t0:t0 + n] = seg[:n] + decb
    return out


# revision 13
# speedup vs baseline: 1.0571x; 1.0258x over previous
"""BitwiseTasNet Trainium2 kernel.

Full (unsharded) inputs in, full output out. Internally: data-parallel over
batch x time across 8 NeuronCores (4 time-shards per batch item) with halo
margins so no inter-core communication is needed. Activations and weights are
bf16 (PSUM accumulation stays fp32). The dilated depthwise conv folds its
center tap into the conv1 eviction (ev1 writes q1 = |w1c|*prelu(bn(conv1)))
so the dconv is one DVE scalar_tensor_tensor (psum = (w2/w1c)*q1R + q1C) plus
one diagonal matmul ((w0/w1c), start=False accumulate); the center-tap sign
folds into ev2's per-channel scale.
"""
import sys

sys.path.insert(0, "/opt/trn_rl_repo")

import numpy as np

import concourse.bass as bass
import concourse.mybir as mybir
import concourse.tile as tile
from concourse.bass_utils import run_bass_kernel_spmd

# Problem constants (hardcoded per contest rules).
B, T, E, D, BL, L, KT, FK, STR = 2, 64000, 256, 512, 2, 6, 3, 20, 10
EPS = 1e-5
TC = (T + 2 * FK - FK) // STR + 1  # 6403 encoder output cols
NCORES, QP = 8, 4  # 4 time-shards per batch item
NI = 1601          # interior cols per core (ceil(6403/4))
MARG = 128         # halo margin (2*63 receptive field + 2 for decoder)
NE = NI + 2 * MARG # 1857 computed cols (block 0 / encoder / decoder)
DOFF = 32          # side strip for dconv tap overhang (max dilation)
BW = 1984          # activation buffer width
XW_LEN = 19240
NL = BL * L
PCOLS_PER_LAYER = 40
NPCOL = NL * PCOLS_PER_LAYER + 8

# PSUM groups are half-width (2 banks, 4 slots) for pipeline depth. Each
# half: matmul segments (psum_off, data_off, width) and an eviction run.
NEW = 1920  # encoder window width (block-0 matmuls span [0, 1920))


def make_halves(a, b):
    """Split the frame interval [a, b) into two psum halves of <=1024 cols,
    each tiled by <=512-col matmul segments at bank-aligned psum offsets."""
    w = b - a
    halves = []
    for off in range(0, w, 1024):
        hw = min(1024, w - off)
        segs = []
        for so in range(0, hw, 512):
            segs.append((so, a + off + so, min(512, hw - so)))
        halves.append(dict(segs=segs, ev=(0, a + off, hw)))
    return halves


def layer_intervals():
    """Per (block, layer): conv1-out interval and dconv/conv2-out interval
    in the NE frame, shrinking with the remaining receptive field."""
    res = {}
    cur = (129, 1731)  # decoder/mask cols
    for bI in reversed(range(BL)):
        for l in reversed(range(L)):
            dil = 1 << l
            out = cur
            c1 = (out[0] - dil, out[1] + dil)
            res[(bI, l)] = (c1, out)
            cur = c1
    return res


INTERVALS = layer_intervals()
# encoder runs in f32r, which requires 128-multiple segment widths at
# bank-aligned psum offsets; evictions still cover exactly [0, NE)
HALVES0 = [
    dict(segs=[(0, 0, 512), (512, 512, 512)], ev=(0, 0, 1024)),
    dict(segs=[(0, 1024, 512), (512, 1536, 384)], ev=(0, 1024, 833)),
]

F32 = mybir.dt.float32
F32R = mybir.dt.float32r
BF16 = mybir.dt.bfloat16
AF = mybir.ActivationFunctionType
OP = mybir.AluOpType

_built = None  # cached (module is data-independent)


def _split_multi_waits(nc, max_waits=1):
    """This walrus build accepts only one sync-wait command per instruction;
    hoist extras into standalone NoOps on the same engine just before it."""
    for fn in nc.m.functions:
        for blk in fn.blocks:
            new_insts, ctr = [], 0
            for inst in blk.instructions:
                si = inst.sync_info
                if si is not None and len(si.on_wait) > max_waits:
                    extra = si.on_wait[:-max_waits]
                    si.on_wait = si.on_wait[-max_waits:]
                    for w in extra:
                        ctr += 1
                        new_insts.append(mybir.InstNoOp(
                            name=f"{inst.name}_hw{ctr}",
                            engine=inst.engine,
                            sync_info=mybir.SyncInfo(on_wait=[w], on_update=[]),
                            bass_nofuse=True,
                        ))
                new_insts.append(inst)
            blk.instructions = new_insts


def build(loop_k=None):
    """Build the (data-independent) bass module for one core."""
    nc = bass.Bass()

    win_d = nc.dram_tensor("win", [FK, NEW], F32R, kind="ExternalInput")
    eye_d = nc.dram_tensor("eye", [128, 128], BF16, kind="ExternalInput")
    mkl_d = nc.dram_tensor("maskL", [128, 64], BF16, kind="ExternalInput")
    mkr_d = nc.dram_tensor("maskR", [128, 64], BF16, kind="ExternalInput")
    par_d = nc.dram_tensor("params", [128, NPCOL], F32, kind="ExternalInput")
    encT_d = nc.dram_tensor("encT", [FK, E], F32R, kind="ExternalInput")
    decT_d = nc.dram_tensor("decT", [128, 2, 20], BF16, kind="ExternalInput")
    w1T_d = nc.dram_tensor("w1T", [NL, 128, 2, D], BF16, kind="ExternalInput")
    w2T_d = nc.dram_tensor("w2T", [NL, 128, 4, E], BF16, kind="ExternalInput")
    y1_d = nc.dram_tensor("y1", [10, NI], F32, kind="ExternalOutput")
    y2_d = nc.dram_tensor("y2", [10, NI], F32, kind="ExternalOutput")

    with tile.TileContext(nc) as tc:
        with (
            tc.tile_pool(name="per", bufs=1) as per,
            tc.tile_pool(name="lw", bufs=3) as lw,
            tc.tile_pool(name="ps", bufs=4, space="PSUM") as psp,
        ):
            # ---- persistent tiles ----
            eye = per.tile([128, 128], BF16)
            mkl = per.tile([128, 64], BF16)
            mkr = per.tile([128, 64], BF16)
            par = per.tile([128, NPCOL], F32)
            encT = per.tile([FK, E], F32R)
            decT = per.tile([128, 2, 20], BF16)
            win = per.tile([FK, NEW], F32R)
            HI0 = per.tile([128, 2, BW], BF16)  # enc / block0 input (preserved)
            HI1 = per.tile([128, 2, BW], BF16)  # block1 input
            hP = per.tile([128, 2, BW], BF16)   # intra-block h scratch
            hF = per.tile([128, 2, BW], BF16)   # final h
            q1 = per.tile([128, 4, BW], BF16)   # |w1c|*prelu1 out (dconv input)
            v = per.tile([128, 4, BW], BF16)    # prelu2 out (conv2 input)
            warm = per.tile([128, 1], F32)

            nc.scalar.dma_start(encT[:], encT_d[:])
            nc.sync.dma_start(win[:, 0:512], win_d[:, 0:512])
            nc.sync.dma_start(win[:, 512:1024], win_d[:, 512:1024])
            nc.sync.dma_start(win[:, 1024:1536], win_d[:, 1024:1536])
            nc.sync.dma_start(win[:, 1536:NEW], win_d[:, 1536:NEW])
            nc.scalar.dma_start(par[:], par_d[:])
            nc.scalar.dma_start(eye[:], eye_d[:])
            nc.gpsimd.dma_start(mkl[:], mkl_d[:])
            nc.gpsimd.dma_start(mkr[:], mkr_d[:])
            nc.gpsimd.dma_start(decT[:], decT_d[:])

            # zero dconv overhang strips of q1 once (never re-written; the
            # computed region [DOFF, DOFF+NE) is re-filled by ev1 each layer)
            for ct in range(4):
                nc.vector.memset(q1[:, ct, 0:DOFF].bitcast(F32), 0.0)
                nc.vector.memset(q1[:, ct, DOFF + NE - 1:BW].bitcast(F32), 0.0)

            # warm the ACT table set early (parametric_relu+sigmoid+identity)
            nc.vector.memset(warm[:], 0.0)
            nc.scalar.activation(warm[:], warm[:], AF.Prelu, bias=0.0, scale=1.0, alpha=0.25)
            nc.scalar.activation(warm[:], warm[:], AF.Sigmoid, bias=0.0, scale=1.0)

            def mm_group(lhts, rhs_of, halves, evict_fn, name, mrows=128,
                         half_outer=False, start=True, pre_fn=None):
                """One output row-tile: matmuls into per-half psum tiles
                (2 banks each) plus per-half evictions. half_outer emits
                [pre, mms, evict] half-by-half so downstream consumers of
                half 0 start earlier; otherwise kt-outer with evictions at
                the end. pre_fn(tile, hi, hv) pre-loads a psum half (then
                start=False accumulates onto it)."""
                tiles = [psp.tile([128, 1024], F32, tag="ps", name=f"{name}{hi}")
                         for hi in range(len(halves))]
                nk = len(lhts)
                if half_outer:
                    for hi, hv in enumerate(halves):
                        if pre_fn is not None:
                            pre_fn(tiles[hi], hi, hv)
                        for ki in range(nk):
                            for (po, do, w) in hv["segs"]:
                                nc.tensor.matmul(
                                    tiles[hi][0:mrows, po:po + w], lhts[ki],
                                    rhs_of(ki, do, w),
                                    start=(ki == 0 and start), stop=(ki == nk - 1),
                                    skip_group_check=True,
                                )
                        (po, do, w) = hv["ev"]
                        evict_fn(tiles[hi], po, do, w)
                else:
                    if pre_fn is not None:
                        for hi, hv in enumerate(halves):
                            pre_fn(tiles[hi], hi, hv)
                    for ki in range(nk):
                        for hi, hv in enumerate(halves):
                            for (po, do, w) in hv["segs"]:
                                nc.tensor.matmul(
                                    tiles[hi][0:mrows, po:po + w], lhts[ki],
                                    rhs_of(ki, do, w),
                                    start=(ki == 0 and start), stop=(ki == nk - 1),
                                    skip_group_check=True,
                                )
                    for hi, hv in enumerate(halves):
                        (po, do, w) = hv["ev"]
                        evict_fn(tiles[hi], po, do, w)

            def emit_body():
                # ---- encoder: enc = encT.T @ win (K=20), evict on DVE ----
                for mt in range(2):
                    ebias = par[:, NL * PCOLS_PER_LAYER + mt: NL * PCOLS_PER_LAYER + mt + 1]
                    def enc_evict(ps, po, do, w, mt=mt, ebias=ebias):
                        nc.vector.tensor_scalar_add(
                            HI0[:, mt, DOFF + do:DOFF + do + w], ps[:, po:po + w],
                            ebias)
                    mm_group(
                        [encT[:, mt * 128:(mt + 1) * 128]],
                        lambda ki, do, w: win[:, do:do + w],
                        HALVES0, enc_evict, f"enc{mt}")

                # ---- TCN ----
                hcur = HI0
                for b in range(BL):
                    resid = hcur
                    for l in range(L):
                        li = b * L + l
                        base = li * PCOLS_PER_LAYER
                        dil = 1 << l
                        (c1a, c1b), (oa, ob) = INTERVALS[(b, l)]
                        hv1 = make_halves(c1a, c1b)
                        hv2 = make_halves(oa, ob)

                        w1t = lw.tile([128, 2, D], BF16, tag="w1t")
                        w2t = lw.tile([128, 4, E], BF16, tag="w2t")
                        dg = lw.tile([128, 4, 128], BF16, tag="dg")
                        nc.sync.dma_start(w1t[:], w1T_d[li])
                        nc.gpsimd.dma_start(w2t[:], w2T_d[li])
                        # diag matrices for the left tap: dg[:, ct, :] = eye*d0
                        for ct in range(4):
                            nc.vector.tensor_scalar_mul(
                                dg[:, ct, :], eye[:],
                                par[:, base + 8 + ct: base + 9 + ct],
                            )

                        # conv1 (E->D) + Prelu/BN eviction into q1 = |w1c|*p
                        def emit_c1(ct, hcur=hcur):
                            def ev1(ps, po, do, w, ct=ct):
                                nc.scalar.activation(
                                    q1[:, ct, DOFF + do:DOFF + do + w], ps[:, po:po + w],
                                    AF.Prelu,
                                    bias=par[:, base + 4 + ct: base + 5 + ct],
                                    scale=par[:, base + ct: base + 1 + ct],
                                    alpha=par[:, base + 38: base + 39],
                                )
                            mm_group(
                                [w1t[:, kt, ct * 128:(ct + 1) * 128] for kt in range(2)],
                                lambda ki, do, w: hcur[:, ki, DOFF + do:DOFF + do + w],
                                hv1, ev1, f"c1_{ct}_", half_outer=True)
                            # zero-pad masks on the dconv input (per-core data),
                            # then fill tap-reachable pad cols with -C1*|w1c| so
                            # the folded dconv bias is exact at true tensor edges
                            nc.vector.tensor_mul(
                                q1[:, ct, 96:160], q1[:, ct, 96:160], mkl[:])
                            nc.vector.tensor_scalar_add(
                                q1[:, ct, 128:160], q1[:, ct, 128:160],
                                par[:, base + 28 + ct: base + 29 + ct])
                            nc.vector.tensor_mul(
                                q1[:, ct, 1760:1824], q1[:, ct, 1760:1824], mkr[:])
                            nc.vector.tensor_scalar_add(
                                q1[:, ct, 1760:1792], q1[:, ct, 1760:1792],
                                par[:, base + 32 + ct: base + 33 + ct])

                        # depthwise dilated conv: DVE stt pre-loads each psum
                        # half with d2*q1R + q1C, then one diagonal matmul
                        # accumulates d0*q1L (start=False); Act evicts with
                        # Prelu/BN (sign of w1c folded into the scale) into v
                        def emit_dc(ct):
                            def pre2(pt, hi, hv, ct=ct):
                                (po, do, w) = hv["ev"]
                                nc.vector.scalar_tensor_tensor(
                                    pt[:, po:po + w],
                                    q1[:, ct, DOFF + dil + do:DOFF + dil + do + w],
                                    par[:, base + 12 + ct: base + 13 + ct],
                                    q1[:, ct, DOFF + do:DOFF + do + w],
                                    op0=OP.mult, op1=OP.add,
                                )
                            def ev2(ps, po, do, w, ct=ct):
                                nc.scalar.activation(
                                    v[:, ct, DOFF + do:DOFF + do + w], ps[:, po:po + w],
                                    AF.Prelu,
                                    bias=par[:, base + 24 + ct: base + 25 + ct],
                                    scale=par[:, base + 20 + ct: base + 21 + ct],
                                    alpha=par[:, base + 39: base + 40],
                                )
                            mm_group(
                                [dg[:, ct, :]],
                                lambda ki, do, w, ct=ct: q1[:, ct, DOFF - dil + do:
                                                            DOFF - dil + do + w],
                                hv2, ev2, f"dc_{ct}_", start=False, pre_fn=pre2,
                                half_outer=True)

                        # conv2 (D->E) + h update
                        last = (l == L - 1)
                        hnext = (HI1 if b == 0 else hF) if last else hP
                        def emit_c2(ct2, resid=resid):
                            eb = par[:, base + 36 + ct2: base + 37 + ct2]
                            def ev3(ps, po, do, w, ct2=ct2, eb=eb, last=last):
                                if last:
                                    nc.vector.scalar_tensor_tensor(
                                        hnext[:, ct2, DOFF + do:DOFF + do + w],
                                        ps[:, po:po + w], eb,
                                        resid[:, ct2, DOFF + do:DOFF + do + w],
                                        op0=OP.add, op1=OP.add,
                                    )
                                else:
                                    nc.vector.tensor_scalar_add(
                                        hnext[:, ct2, DOFF + do:DOFF + do + w],
                                        ps[:, po:po + w], eb)
                            mm_group(
                                [w2t[:, kt, ct2 * 128:(ct2 + 1) * 128] for kt in range(4)],
                                lambda ki, do, w: v[:, ki, DOFF + do:DOFF + do + w],
                                hv2, ev3, f"c2_{ct2}_", half_outer=True)

                        # interleave so each dconv's psum tiles allocate right
                        # after that ct's conv1 tiles release (stt starts at
                        # ev1-ct, not ev1-ct+2), and conv2 follows
                        emit_c1(0); emit_c1(1); emit_dc(0)
                        emit_c1(2); emit_dc(1)
                        emit_c1(3); emit_dc(2); emit_dc(3)
                        emit_c2(0); emit_c2(1)
                        hcur = hnext

                # ---- mask + decoder (cols [128, 1731) only) ----
                sig = q1  # reuse
                mk = v
                for ct2 in range(2):
                    for (c0, c1) in ((129, 529), (529, 929), (929, 1331), (1331, 1731)):
                        nc.scalar.activation(
                            sig[:, ct2, DOFF + c0:DOFF + c1], hF[:, ct2, DOFF + c0:DOFF + c1],
                            AF.Sigmoid, bias=0.0, scale=1.0)
                        nc.vector.tensor_mul(
                            mk[:, ct2, DOFF + c0:DOFF + c1],
                            HI0[:, ct2, DOFF + c0:DOFF + c1],
                            sig[:, ct2, DOFF + c0:DOFF + c1])
                dsb = per.tile([20, NE], F32)
                def evd(ps, po, do, w):
                    nc.scalar.activation(
                        dsb[:, do:do + w], ps[0:20, po:po + w], AF.Copy)
                mm_group(
                    [decT[:, kt, :] for kt in range(2)],
                    lambda ki, do, w: mk[:, ki, DOFF + do:DOFF + do + w],
                    make_halves(129, 1731), evd, "dec_", mrows=20, half_outer=True)
                # out[10m+r] = P1[r, m+MARG+2] + P2[r, m+MARG+1]  (host adds them)
                PB = 129 + 1024  # dsb piece-A boundary
                ys = PB - MARG - 2  # first-piece width aligned to dsb piece A
                nc.sync.dma_start(y1_d[:, 0:ys], dsb[0:10, MARG + 2:PB])
                nc.sync.dma_start(y2_d[:, 0:ys + 1], dsb[10:20, MARG + 1:PB])
                nc.sync.dma_start(y1_d[:, ys:NI], dsb[0:10, PB:MARG + 2 + NI])
                nc.sync.dma_start(y2_d[:, ys + 1:NI], dsb[10:20, PB:MARG + 1 + NI])


            if loop_k is None:
                emit_body()
            else:
                with tc.For_i(0, loop_k):
                    emit_body()

    _split_multi_waits(nc)
    return nc


def _host_prep(inputs):
    """Per-core in_maps + assembly metadata from full inputs."""
    import ml_dtypes
    f32 = np.float32
    bf16 = ml_dtypes.bfloat16
    x = np.asarray(inputs["x"], f32)
    enc_w = np.asarray(inputs["enc_w"], f32)
    enc_b = np.asarray(inputs["enc_b"], f32)
    w1 = np.asarray(inputs["w1"], f32)
    b1 = np.asarray(inputs["b1"], f32)
    a1 = np.asarray(inputs["a1"], f32)
    g1 = np.asarray(inputs["g1"], f32)
    be1 = np.asarray(inputs["be1"], f32)
    m1 = np.asarray(inputs["m1"], f32)
    v1 = np.asarray(inputs["v1"], f32)
    wd = np.asarray(inputs["wd"], f32)
    bd = np.asarray(inputs["bd"], f32)
    a2 = np.asarray(inputs["a2"], f32)
    g2 = np.asarray(inputs["g2"], f32)
    be2 = np.asarray(inputs["be2"], f32)
    m2 = np.asarray(inputs["m2"], f32)
    v2 = np.asarray(inputs["v2"], f32)
    w2 = np.asarray(inputs["w2"], f32)
    b2 = np.asarray(inputs["b2"], f32)
    dec_w = np.asarray(inputs["dec_w"], f32)
    dec_b = np.asarray(inputs["dec_b"], f32)

    eye = np.eye(128, dtype=f32).astype(bf16)
    encT = np.ascontiguousarray(enc_w[:, 0, :].T)  # [FK, E]
    decT = np.zeros((128, 2, 20), f32)
    for kt in range(2):
        decT[:, kt, :] = dec_w[kt * 128:(kt + 1) * 128, 0, :]
    decT = decT.astype(bf16)

    w1T = np.zeros((NL, 128, 2, D), f32)
    w2T = np.zeros((NL, 128, 4, E), f32)
    C1s = np.zeros((NL, D), np.float64)  # (-c1)*|w1c| edge-fix values
    par_shared = np.zeros((128, NPCOL), f32)
    for b in range(BL):
        for l in range(L):
            li = b * L + l
            base = li * PCOLS_PER_LAYER
            w1bl = w1[b, l, :, :, 0].astype(np.float64)  # [D, E]
            w2bl = w2[b, l, :, :, 0].astype(np.float64)  # [E, D]
            for kt in range(2):
                w1T[li, :, kt, :] = w1bl.T[kt * 128:(kt + 1) * 128, :]
            for kt in range(4):
                w2T[li, :, kt, :] = w2bl.T[kt * 128:(kt + 1) * 128, :]
            s1 = g1[b, l] / np.sqrt(v1[b, l].astype(np.float64) + EPS)
            c1 = be1[b, l] - m1[b, l] * s1
            s2 = g2[b, l] / np.sqrt(v2[b, l].astype(np.float64) + EPS)
            c2 = be2[b, l] - m2[b, l] * s2
            w0, w1c, w2c = (wd[b, l, :, 0, k].astype(np.float64) for k in range(3))
            aw = np.abs(w1c)
            sg = np.sign(w1c)
            C1s[li] = (-c1) * aw
            bias2p = bd[b, l] + c1 * (w0 + w1c + w2c)
            b2pp = b2[b, l] + w2bl @ c2

            def col(idx, vals512):
                par_shared[:, base + idx:base + idx + 4] = np.asarray(
                    vals512, f32).reshape(4, 128).T
            col(0, s1 * aw)
            col(4, s1 * b1[b, l] * aw)
            col(8, w0 / w1c)   # d0 (left-tap diag values)
            col(12, w2c / w1c) # d2 (right-tap stt scale)
            col(20, s2 * sg)
            col(24, s2 * bias2p)
            # 28..35: fixL/fixR are per-core (filled later)
            par_shared[:, base + 36:base + 38] = np.asarray(
                b2pp, f32).reshape(2, 128).T
            par_shared[:, base + 38] = a1[b, l]
            par_shared[:, base + 39] = a2[b, l]
    par_shared[:, NL * PCOLS_PER_LAYER:NL * PCOLS_PER_LAYER + 2] = \
        enc_b.reshape(2, 128).T

    w1T = w1T.astype(bf16)
    w2T = w2T.astype(bf16)

    in_maps = []
    ones64 = np.ones((128, 64), f32).astype(bf16)
    zeros64 = np.zeros((128, 64), f32).astype(bf16)
    for core in range(NCORES):
        bb, q = divmod(core, QP)
        xbase = 16010 * q - 1300
        xw = np.zeros(XW_LEN, f32)
        lo, hi = max(0, xbase), min(T, xbase + XW_LEN)
        if hi > lo:
            xw[lo - xbase:hi - xbase] = x[bb, 0, lo:hi]
        # im2col on host: win[k, j] = xw[10j + k]
        winm = np.lib.stride_tricks.as_strided(
            xw, shape=(1920, FK), strides=(40, 4)).T.copy()
        par = par_shared.copy()
        left, right = (q == 0), (q == QP - 1)
        for li in range(NL):
            base = li * PCOLS_PER_LAYER
            fixL = C1s[li] if left else np.zeros(D)
            fixR = C1s[li] if right else np.zeros(D)
            par[:, base + 28:base + 32] = np.asarray(fixL, f32).reshape(4, 128).T
            par[:, base + 32:base + 36] = np.asarray(fixR, f32).reshape(4, 128).T
        in_maps.append(dict(
            win=winm, eye=eye,
            maskL=(zeros64 if left else ones64),
            maskR=(zeros64 if right else ones64),
            params=par, encT=encT, decT=decT, w1T=w1T, w2T=w2T,
        ))
    return in_maps, float(dec_b[0])


def kernel(**inputs):
    global _built
    if _built is None:
        _built = build()
    nc = _built
    in_maps, decb = _host_prep(inputs)
    res = run_bass_kernel_spmd(nc, in_maps, core_ids=list(range(NCORES)))
    out = np.zeros((B, 1, T), np.float32)
    for core in range(NCORES):
        bb, q = divmod(core, QP)
        seg = (res.results[core]["y1"] + res.results[core]["y2"]).T.reshape(-1)
        t0 = q * NI * STR
        n = min(T - t0, NI * STR)
        out[bb, 0, t0:t0 + n] = seg[:n] + decb
    return out
